# revision 13
# baseline (speedup 1.0000x reference)
"""YOLOv1 loss kernel v2 for 8 Trainium2 NeuronCores.

Data parallel over batch (2048 per core, 100352 cells), tiled T=2 x
[P=128, F=392] cells. Six input streams per core, each a contiguous DRAM
tensor so every DMA moves whole partition lines (planar field layout):
  - boxa: 8 bf16 planes (pred/target xy) - lands first, feeds the subs
  - boxb: 12 bf16 planes (wh / conf / coo / t9)
  - clsa/clsb: 40 fp8e4m3 class planes split 192+200 cells, so the mask
    AND + PE matmuls on half A overlap half B's DMA
  - cmska/cmskb: coo mask as 0xFF/0x00 bytes, u16-paired

Engine split:
  - class loss on TensorE: class = sum coo*(p-t)^2 expands to diag combos of
    the Gram matrix G = U^T (coo*U) with U = [p_cls | t_cls].  The coo mask
    is applied as a bitwise AND on fp8 *pairs* reinterpreted as u16 (2x DVE
    mode), and G accumulates in PSUM over 392 fp8-DoubleRow matmuls (k-tile
    pairs are cells 16 apart per the LDW dual-fp8 ISA stride rule) + 16
    plain tail matmuls.
  - box losses on DVE (tensor_tensor 2x bf16; broadcast APs keep the outer
    dim at stride 0 and the inner at stride 1 so 2x_1p still engages) + ACT
    (Abs/Sqrt/Square from the single sqrt_and_others table set, each with
    free scale and accum_out).  1/union uses a bf16 bit-magic reciprocal
    (0x7EF2 - bits) plus one Newton step on DVE: Act.Reciprocal is banned
    in bass and vector.reciprocal costs ~8 cyc/elem.
  - every masked partial sum uses the binary-mask trick m = mask*value with
    mask^2 = mask, so ACT Square(scale*m) with accum_out folds the final
    loss weight (5/2/1/0.5) into scale and emits the column sum for free.

Host side only shards, transposes to planar layout, and casts dtypes; all
arithmetic on pred/target happens on device.
"""

import sys

if "/opt/trn_rl_repo" not in sys.path:
    sys.path.insert(0, "/opt/trn_rl_repo")

import numpy as np
import ml_dtypes

import concourse.bass as bass
import concourse.tile as tile
from concourse import mybir
from concourse.bass_utils import run_bass_kernel_spmd

BF16 = ml_dtypes.bfloat16
FP8 = ml_dtypes.float8_e4m3fn

NCORES = 8
B, S, C = 16384, 7, 30
BS = B // NCORES            # 2048 batches per core
NCELL = BS * S * S          # 100352 cells per core
P = 128
T = 2
F = NCELL // (T * P)        # 392 cells per partition-row per tile
INV = 1.0 / 14.0
NACC = 4                    # accum columns per tile: loc, contain, ncont, noo

f32 = mybir.dt.float32
bf16 = mybir.dt.bfloat16
u16 = mybir.dt.uint16
fp8 = mybir.dt.float8e4
Alu = mybir.AluOpType
Act = mybir.ActivationFunctionType

SQ5 = float(np.sqrt(5.0))
SQ2 = float(np.sqrt(2.0))
SQH = float(np.sqrt(0.5))


def build_nc(fix_waits=True, repeat=0, level=4):
    nc = bass.Bass(target_bir_lowering=False)
    # box stream split: xy planes (feed the first DVE subs) and the rest
    boxa = nc.declare_dram_parameter("boxa", [T * P, 8, F], bf16, isOutput=False)
    boxb = nc.declare_dram_parameter("boxb", [T * P, 12, F], bf16, isOutput=False)
    # class stream split into two contiguous halves (192 + 200 cells) so
    # masking + PE matmuls on half A overlap the DMA of half B
    FA, FB = 192, F - 192
    clsa = nc.declare_dram_parameter("clsa", [T * P, 40, FA], fp8, isOutput=False)
    clsb = nc.declare_dram_parameter("clsb", [T * P, 40, FB], fp8, isOutput=False)
    cmska = nc.declare_dram_parameter("cmska", [T * P, FA // 2], u16, isOutput=False)
    cmskb = nc.declare_dram_parameter("cmskb", [T * P, FB // 2], u16, isOutput=False)
    out = nc.declare_dram_parameter("out", [P, T * NACC], f32, isOutput=True)
    gout = nc.declare_dram_parameter("gout", [40, 40], f32, isOutput=True)

    V = nc.vector
    A = nc.scalar
    G = nc.gpsimd

    with tile.TileContext(nc) as tc:
        with tc.tile_pool(name="io", bufs=2) as io, \
             tc.tile_pool(name="tmp", bufs=2) as tmp, \
             tc.tile_pool(name="accp", bufs=1) as accp, \
             tc.tile_pool(name="psp", bufs=1, space="PSUM") as psp:
            acc = accp.tile([P, T * NACC], f32)
            V.memset(acc, 0.0)
            kpl = accp.tile([P, F], u16)
            V.memset(kpl, 0x7EF2)
            gps = psp.tile([40, 40], f32)
            gsb = accp.tile([40, 40], f32)

            import contextlib
            rep_ctx = tc.For_i(0, repeat, 1) if repeat else contextlib.nullcontext()
            with rep_ctx:
                for it in range(T):
                    r0, r1 = it * P, (it + 1) * P
                    bta = io.tile([P, 8, F], bf16, tag="bta")
                    btb = io.tile([P, 12, F], bf16, tag="btb")
                    ua = io.tile([P, 40, FA], fp8, tag="ua")
                    ub = io.tile([P, 40, FB], fp8, tag="ub")
                    cma = io.tile([P, FA // 2], u16, tag="cma")
                    cmb_t = io.tile([P, FB // 2], u16, tag="cmb")
                    subs = [(ua, cma, clsa, cmska, FA, "va"),
                            (ub, cmb_t, clsb, cmskb, FB, "vb")]
                    if level != 5:
                        # tile 0: class stream first (PE chain starts early);
                        # last tile: box first, so the box-compute tail and
                        # the PE tail overlap instead of stacking.
                        if it < T - 1:
                            nc.sync.dma_start(out=cma, in_=cmska[r0:r1])
                            nc.sync.dma_start(out=ua, in_=clsa[r0:r1])
                            nc.sync.dma_start(out=bta, in_=boxa[r0:r1])
                            nc.sync.dma_start(out=btb, in_=boxb[r0:r1])
                            nc.sync.dma_start(out=cmb_t, in_=cmskb[r0:r1])
                            nc.sync.dma_start(out=ub, in_=clsb[r0:r1])
                        else:
                            nc.sync.dma_start(out=bta, in_=boxa[r0:r1])
                            nc.sync.dma_start(out=cma, in_=cmska[r0:r1])
                            nc.sync.dma_start(out=ua, in_=clsa[r0:r1])
                            nc.sync.dma_start(out=btb, in_=boxb[r0:r1])
                            nc.sync.dma_start(out=cmb_t, in_=cmskb[r0:r1])
                            nc.sync.dma_start(out=ub, in_=clsb[r0:r1])
                    if level == 0:
                        continue

                    # ---- class block: V = coo-mask AND U; G += U^T V ----
                    # AND is chunked per 32-cell block so the PE can start on
                    # block 0 while later blocks are still being masked.
                    for si, (uS, cS, dU, dC, nS, vtag) in enumerate(subs):
                        vS = tmp.tile([P, 40, nS], fp8, tag=vtag)
                        NBs = nS // 32
                        for bb in range((nS + 63) // 64):
                            c0, c1 = 64 * bb, min(64 * (bb + 1), nS)
                            cmv = cS[:, c0 // 2:c1 // 2].unsqueeze(1) \
                                .broadcast_to([P, 40, (c1 - c0) // 2])
                            V.tensor_tensor(vS[:, :, c0:c1].bitcast(u16),
                                            uS[:, :, c0:c1].bitcast(u16), cmv,
                                            op=Alu.bitwise_and)
                        if level == 6:
                            continue
                        # DoubleRow fp8: k-tile pairs are cells 16 apart (the
                        # LDW dual-fp8 ISA rule needs ktile step % 16 == 0).
                        for bb in range(NBs):
                            for j in range(16):
                                o = 32 * bb + j
                                lhs = uS[:, :, o:o + 17:16].rearrange(
                                    "p c k -> p k c", k=2)
                                rhs = vS[:, :, o:o + 17:16].rearrange(
                                    "p c k -> p k c", k=2)
                                nc.tensor.matmul(
                                    gps[0:40, 0:40], lhs, rhs,
                                    start=(it == 0 and si == 0 and bb == 0
                                           and j == 0),
                                    stop=False,
                                    perf_mode=mybir.MatmulPerfMode.DoubleRow)
                        for f in range(32 * NBs, nS):
                            nc.tensor.matmul(
                                gps[0:40, 0:40], uS[:, :, f], vS[:, :, f],
                                start=False,
                                stop=(it == T - 1 and si == 1 and f == nS - 1))
                    if level == 1 or level == 6:
                        continue

                    # ---- box block ----
                    pxy4 = bta[:, 0:4]    # cx0 cy0 cx1 cy1
                    txy4 = bta[:, 4:8]    # tx ty tx1 ty1
                    pwh4 = btb[:, 0:4]    # w0 h0 w1 h1
                    twh4 = btb[:, 4:8]    # tw th tw1 th1
                    twh01 = btb[:, 4:6]   # tw th
                    pc01 = btb[:, 8:10]   # c0 c1
                    coo = btb[:, 10]
                    tc01 = btb[:, 10:12]  # coo t9

                    scr = tmp.tile([P, 50, F], bf16, tag="scr")
                    b = it * NACC

                    twhb = twh01.rearrange("p a f -> p (a f)").unsqueeze(1) \
                        .broadcast_to([P, 2, 2 * F])
                    pwh22 = pwh4.rearrange("p (a b) f -> p a (b f)", a=2, b=2)

                    # center diffs: d00 | d11 | d10 (planes 0-5)
                    V.tensor_sub(scr[:, 0:4], pxy4, txy4)
                    V.tensor_sub(scr[:, 4:6], pxy4[:, 2:4], txy4[:, 0:2])
                    # conf diffs d49 (planes 6,7)
                    V.tensor_sub(scr[:, 6:8], pc01, tc01)

                    # av4 = |(d00,d10)|/14  (planes 8-11)
                    div = scr[:, 0:6].rearrange("p (a b) f -> p a (b f)", a=3, b=2)
                    avin = div[:, 0:3:2]
                    av4 = scr[:, 8:12].rearrange("p (a b) f -> p a (b f)", a=2, b=2)
                    A.activation(av4, avin, Act.Abs, scale=INV)

                    # s4 = wp+wt, A4 = 0.5*s4 (planes 12-15)
                    s4 = scr[:, 12:16]
                    s422 = s4.rearrange("p (a b) f -> p a (b f)", a=2, b=2)
                    V.tensor_tensor(s422, pwh22, twhb, op=Alu.add)
                    V.tensor_scalar(out=s4, in0=s4, scalar1=0.5, scalar2=None,
                                    op0=Alu.mult)
                    # oh4 = relu(min(A4-av4, wp, wt)) (planes 8-11)
                    oh4 = scr[:, 8:12]
                    oh422 = oh4.rearrange("p (a b) f -> p a (b f)", a=2, b=2)
                    V.tensor_sub(oh4, s4, oh4)
                    V.tensor_tensor(oh4, oh4, pwh4, op=Alu.min)
                    V.tensor_tensor(oh422, oh422, twhb, op=Alu.min)
                    V.tensor_scalar(out=oh4, in0=oh4, scalar1=0.0, scalar2=None,
                                    op0=Alu.max)
                    # inter (16,17), areas (18,19,20)
                    ohxy = scr[:, 8:12].rearrange("p (a b) f -> p b a f", a=2, b=2)
                    V.tensor_mul(scr[:, 16:18], ohxy[:, 0], ohxy[:, 1])
                    pwhxy = pwh4.rearrange("p (a b) f -> p b a f", a=2, b=2)
                    V.tensor_mul(scr[:, 18:20], pwhxy[:, 0], pwhxy[:, 1])
                    V.tensor_mul(scr[:, 20], btb[:, 4], btb[:, 5])
                    # union (21,22) = ap - inter + at
                    V.tensor_sub(scr[:, 21:23], scr[:, 18:20], scr[:, 16:18])
                    atb = scr[:, 20].unsqueeze(1).broadcast_to([P, 2, F])
                    V.tensor_tensor(scr[:, 21:23], scr[:, 21:23], atb, op=Alu.add)
                    # iou = inter / union: bit-magic recip + 1 Newton step
                    kb = kpl[:, :].unsqueeze(1).broadcast_to([P, 2, F])
                    r0 = scr[:, 23:25]
                    V.tensor_tensor(r0.bitcast(u16), kb,
                                    scr[:, 21:23].bitcast(u16), op=Alu.subtract)
                    tn = scr[:, 25:27]
                    V.tensor_mul(tn, scr[:, 21:23], r0)
                    V.tensor_scalar(out=tn, in0=tn, scalar1=-1.0, scalar2=2.0,
                                    op0=Alu.mult, op1=Alu.add)
                    V.tensor_mul(r0, r0, tn)
                    V.tensor_mul(scr[:, 16:18], scr[:, 16:18], r0)
                    if level == 2:
                        continue
                    # sel (27), miou (28)
                    V.tensor_tensor(scr[:, 27], scr[:, 17], scr[:, 16],
                                    op=Alu.is_gt)
                    V.tensor_tensor(scr[:, 28], scr[:, 17], scr[:, 16],
                                    op=Alu.max)
                    # masks cs0 (29), cs1 (30)
                    V.tensor_mul(scr[:, 30], coo, scr[:, 27])
                    V.tensor_sub(scr[:, 29], coo, scr[:, 30])
                    cs01 = scr[:, 29:31]
                    csdup = cs01.unsqueeze(2).broadcast_to([P, 2, 2, F])

                    # contain: (c - miou)^2 masked -> col b+1, weight 2
                    mioub = scr[:, 28].unsqueeze(1).broadcast_to([P, 2, F])
                    V.tensor_sub(scr[:, 31:33], pc01, mioub)
                    V.tensor_mul(scr[:, 33:35], cs01, scr[:, 31:33])
                    A.activation(scr[:, 31:33], scr[:, 33:35], Act.Square,
                                 scale=SQ2, accum_out=acc[:, b + 1:b + 2])
                    # not-contain: conf of non-responsible box -> col b+2, w 1
                    V.tensor_mul(scr[:, 35], scr[:, 30], btb[:, 8])
                    V.tensor_mul(scr[:, 36], scr[:, 29], btb[:, 9])
                    A.activation(scr[:, 33:35], scr[:, 35:37], Act.Square,
                                 accum_out=acc[:, b + 2:b + 3])
                    # nooobj: noo*(d4^2+d9^2) -> col b+3, weight 0.5
                    V.tensor_scalar(out=scr[:, 37], in0=coo, scalar1=-1.0,
                                    scalar2=1.0, op0=Alu.mult, op1=Alu.add)
                    noob = scr[:, 37].unsqueeze(1).broadcast_to([P, 2, F])
                    V.tensor_mul(scr[:, 6:8], noob, scr[:, 6:8])
                    A.activation(scr[:, 35:37], scr[:, 6:8], Act.Square,
                                 scale=SQH, accum_out=acc[:, b + 3:b + 4])
                    if level == 3:
                        continue

                    # loc: xy part masked (38-41)
                    dloc = scr[:, 0:4].rearrange("p (a b) f -> p a b f", a=2, b=2)
                    mdxy = scr[:, 38:42].rearrange("p (a b) f -> p a b f", a=2, b=2)
                    V.tensor_tensor(mdxy, csdup, dloc, op=Alu.mult)
                    # loc: wh part (sqrt p - sqrt t) masked (42-45)
                    A.activation(scr[:, 12:20], btb[:, 0:8], Act.Sqrt)
                    V.tensor_sub(scr[:, 12:16], scr[:, 12:16], scr[:, 16:20])
                    ds22 = scr[:, 12:16].rearrange("p (a b) f -> p a b f", a=2, b=2)
                    mds = scr[:, 42:46].rearrange("p (a b) f -> p a b f", a=2, b=2)
                    V.tensor_tensor(mds, csdup, ds22, op=Alu.mult)
                    # single masked-square accum over planes 38-45 -> col b, w 5
                    A.activation(scr[:, 12:20], scr[:, 38:46], Act.Square,
                                 scale=SQ5, accum_out=acc[:, b + 0:b + 1])

            if level >= 1 and level != 6:
                A.copy(gsb, gps)
            else:
                V.memset(gsb, 0.0)
            G.dma_start(out=out[:, :], in_=acc[:, :])
            G.dma_start(out=gout[:, :], in_=gsb)
    if fix_waits:
        _fix_multi_waits(nc)
    return nc


def _fix_multi_waits(nc):
    """Split multi-wait instructions (walrus allows one sync-wait per inst)."""
    import concourse.mybir as _mybir
    from bass_rust import SyncInfo

    blocks = [bb for fn in nc.m.functions for bb in fn.blocks]
    nseq = [0]

    def make_wait(eng, w):
        nseq[0] += 1
        ev = _mybir.InstEventSemaphore(name=f"W-split-{nseq[0]}")
        ev.engine = eng
        ev.sync_info = SyncInfo(on_wait=[w], on_update=[])
        return ev

    for bb in blocks:
        i = 0
        while i < len(bb.instructions):
            inst = bb.instructions[i]
            eng = getattr(inst, "engine", None)
            si = inst.sync_info
            if eng is None or si is None or len(si.on_wait) < 2:
                i += 1
                continue
            waits = list(si.on_wait)
            while len(waits) > 1:
                bb.instructions.insert(i, make_wait(eng, waits.pop(0)))
                i += 1
            si.on_wait[:] = waits
            i += 1


def make_in_maps(pred, target, ncores=NCORES):
    """Shard + repack host side. pred/target: [B,S,S,C] f32 np arrays."""
    bs = pred.shape[0] // ncores
    # plane index -> source (0=pred,1=target), channel
    box_src = [(0, 0), (0, 1), (0, 5), (0, 6),
               (1, 0), (1, 1), (1, 5), (1, 6),
               (0, 2), (0, 3), (0, 7), (0, 8),
               (1, 2), (1, 3), (1, 7), (1, 8),
               (0, 4), (0, 9),
               (1, 4), (1, 9)]
    in_maps = []
    for i in range(ncores):
        pf = pred[i * bs:(i + 1) * bs].reshape(-1, C)
        tf = target[i * bs:(i + 1) * bs].reshape(-1, C)
        src = (pf, tf)
        boxp = np.stack([src[s][:, c] for s, c in box_src], axis=0)  # [20, N]
        boxp = boxp.reshape(20, T * P, F).transpose(1, 0, 2).astype(BF16)
        clsp = np.concatenate([pf[:, 10:30].T, tf[:, 10:30].T], axis=0)
        clsp = clsp.reshape(40, T * P, F).transpose(1, 0, 2).astype(FP8)
        coo = (tf[:, 4] > 0)
        cm8 = np.where(coo, 0xFF, 0).astype(np.uint8)
        cm = cm8.reshape(T * P, F // 2, 2).view(np.uint16)[:, :, 0]
        in_maps.append({
            "boxa": np.ascontiguousarray(boxp[:, 0:8]),
            "boxb": np.ascontiguousarray(boxp[:, 8:20]),
            "clsa": np.ascontiguousarray(clsp[:, :, 0:192]),
            "clsb": np.ascontiguousarray(clsp[:, :, 192:]),
            "cmska": np.ascontiguousarray(cm[:, 0:96]),
            "cmskb": np.ascontiguousarray(cm[:, 96:]),
        })
    return in_maps


def combine(outs, n):
    """outs: list of per-core dicts with 'out' [P,T*NACC] and 'gout' [40,40]."""
    loc = contain = ncont = noo = cls_sum = 0.0
    for o in outs:
        a = o["out"].astype(np.float64).reshape(P, T, NACC).sum(axis=(0, 1))
        loc += a[0] / 5.0
        contain += a[1] / 2.0
        ncont += a[2]
        noo += a[3] / 0.5
        g = o["gout"].astype(np.float64)
        d = np.arange(20)
        cls_sum += (g[d, d].sum() + g[d + 20, d + 20].sum()
                    - 2.0 * g[d, d + 20].sum())
    total = (5.0 * loc + 2.0 * contain + ncont + 0.5 * noo + cls_sum) / n
    return (np.float32(total), np.float32(loc), np.float32(contain),
            np.float32(noo), np.float32(cls_sum))


_NC_CACHE = {}


def _get_nc():
    if "nc" not in _NC_CACHE:
        _NC_CACHE["nc"] = build_nc()
    return _NC_CACHE["nc"]


def run(in_maps, nc=None, **kw):
    if nc is None:
        nc = _get_nc()
    return run_bass_kernel_spmd(nc, in_maps, core_ids=list(range(len(in_maps))), **kw)


def kernel(pred, target):
    pred = np.asarray(pred, dtype=np.float32)
    target = np.asarray(target, dtype=np.float32)
    in_maps = make_in_maps(pred, target)
    res = run(in_maps)
    return combine(res.results, pred.shape[0])


if __name__ == "__main__":
    rng = np.random.default_rng(0)
    pred = rng.uniform(0.01, 1.0, (B, S, S, C)).astype(np.float32)
    target = rng.uniform(0.01, 1.0, (B, S, S, C)).astype(np.float32)
    target[..., 4] = (rng.uniform(size=(B, S, S)) < 0.1).astype(np.float32)
    print(kernel(pred, target))


# revision 14
# speedup vs baseline: 1.0315x; 1.0315x over previous
"""YOLOv1 loss kernel v2 for 8 Trainium2 NeuronCores.

Data parallel over batch (2048 per core, 100352 cells), tiled T=2 x
[P=128, F=392] cells. Six input streams per core, each a contiguous DRAM
tensor so every DMA moves whole partition lines (planar field layout):
  - boxa: 8 bf16 planes (pred/target xy) - lands first, feeds the subs
  - boxb: 12 bf16 planes (wh / conf / coo / t9)
  - clsa/clsb: 40 fp8e4m3 class planes split 192+200 cells, so the mask
    AND + PE matmuls on half A overlap half B's DMA
  - cmska/cmskb: coo mask as 0xFF/0x00 bytes, u16-paired

Engine split:
  - class loss on TensorE: class = sum coo*(p-t)^2 expands to diag combos of
    the Gram matrix G = U^T (coo*U) with U = [p_cls | t_cls].  The coo mask
    is applied as a bitwise AND on fp8 *pairs* reinterpreted as u16 (2x DVE
    mode), and G accumulates in PSUM over 392 fp8-DoubleRow matmuls (k-tile
    pairs are cells 16 apart per the LDW dual-fp8 ISA stride rule) + 16
    plain tail matmuls.
  - box losses on DVE (tensor_tensor 2x bf16; broadcast APs keep the outer
    dim at stride 0 and the inner at stride 1 so 2x_1p still engages) + ACT
    (Abs/Sqrt/Square from the single sqrt_and_others table set, each with
    free scale and accum_out).  1/union uses a bf16 bit-magic reciprocal
    (0x7EF2 - bits) plus one Newton step on DVE: Act.Reciprocal is banned
    in bass and vector.reciprocal costs ~8 cyc/elem.
  - every masked partial sum uses the binary-mask trick m = mask*value with
    mask^2 = mask, so ACT Square(scale*m) with accum_out folds the final
    loss weight (5/2/1/0.5) into scale and emits the column sum for free.

Host side only shards, transposes to planar layout, and casts dtypes; all
arithmetic on pred/target happens on device.
"""

import sys

if "/opt/trn_rl_repo" not in sys.path:
    sys.path.insert(0, "/opt/trn_rl_repo")

import numpy as np
import ml_dtypes

import concourse.bass as bass
import concourse.tile as tile
from concourse import mybir
from concourse.bass_utils import run_bass_kernel_spmd

BF16 = ml_dtypes.bfloat16
FP8 = ml_dtypes.float8_e4m3fn

NCORES = 8
B, S, C = 16384, 7, 30
BS = B // NCORES            # 2048 batches per core
NCELL = BS * S * S          # 100352 cells per core
P = 128
T = 2
F = NCELL // (T * P)        # 392 cells per partition-row per tile
INV = 1.0 / 14.0
NACC = 4                    # accum columns per tile: loc, contain, ncont, noo

f32 = mybir.dt.float32
bf16 = mybir.dt.bfloat16
u16 = mybir.dt.uint16
fp8 = mybir.dt.float8e4
Alu = mybir.AluOpType
Act = mybir.ActivationFunctionType

SQ5 = float(np.sqrt(5.0))
SQ2 = float(np.sqrt(2.0))
SQH = float(np.sqrt(0.5))


def build_nc(fix_waits=True, repeat=0, level=4):
    nc = bass.Bass(target_bir_lowering=False)
    # box stream split: xy planes (feed the first DVE subs) and the rest
    boxa = nc.declare_dram_parameter("boxa", [T * P, 8, F], bf16, isOutput=False)
    boxb = nc.declare_dram_parameter("boxb", [T * P, 12, F], bf16, isOutput=False)
    # class stream split into two contiguous halves (192 + 200 cells) so
    # masking + PE matmuls on half A overlap the DMA of half B
    FA, FB = 192, F - 192
    clsa = nc.declare_dram_parameter("clsa", [T * P, 40, FA], fp8, isOutput=False)
    clsb = nc.declare_dram_parameter("clsb", [T * P, 40, FB], fp8, isOutput=False)
    cmska = nc.declare_dram_parameter("cmska", [T * P, FA // 2], u16, isOutput=False)
    cmskb = nc.declare_dram_parameter("cmskb", [T * P, FB // 2], u16, isOutput=False)
    out = nc.declare_dram_parameter("out", [P, T * NACC], f32, isOutput=True)
    gout = nc.declare_dram_parameter("gout", [40, 40], f32, isOutput=True)

    V = nc.vector
    A = nc.scalar
    G = nc.gpsimd

    with tile.TileContext(nc) as tc:
        with tc.tile_pool(name="io", bufs=2) as io, \
             tc.tile_pool(name="tmp", bufs=2) as tmp, \
             tc.tile_pool(name="accp", bufs=1) as accp, \
             tc.tile_pool(name="psp", bufs=1, space="PSUM") as psp:
            acc = accp.tile([P, T * NACC], f32)
            V.memset(acc, 0.0)
            kpl = accp.tile([P, F], u16)
            V.memset(kpl, 0x7EF2)
            gps = psp.tile([40, 40], f32)
            gsb = accp.tile([40, 40], f32)

            import contextlib
            rep_ctx = tc.For_i(0, repeat, 1) if repeat else contextlib.nullcontext()
            with rep_ctx:
                for it in range(T):
                    r0, r1 = it * P, (it + 1) * P
                    bta = io.tile([P, 8, F], bf16, tag="bta")
                    btb = io.tile([P, 12, F], bf16, tag="btb")
                    ua = io.tile([P, 40, FA], fp8, tag="ua")
                    ub = io.tile([P, 40, FB], fp8, tag="ub")
                    cma = io.tile([P, FA // 2], u16, tag="cma")
                    cmb_t = io.tile([P, FB // 2], u16, tag="cmb")
                    subs = [(ua, cma, clsa, cmska, FA, "va"),
                            (ub, cmb_t, clsb, cmskb, FB, "vb")]
                    if level != 5:
                        # tile 0: class stream first (PE chain starts early);
                        # last tile: box first, so the box-compute tail and
                        # the PE tail overlap instead of stacking.
                        if it < T - 1:
                            nc.sync.dma_start(out=cma, in_=cmska[r0:r1])
                            nc.sync.dma_start(out=ua, in_=clsa[r0:r1])
                            nc.sync.dma_start(out=bta, in_=boxa[r0:r1])
                            nc.sync.dma_start(out=cmb_t, in_=cmskb[r0:r1])
                            nc.sync.dma_start(out=ub, in_=clsb[r0:r1])
                            nc.sync.dma_start(out=btb, in_=boxb[r0:r1])
                        else:
                            nc.sync.dma_start(out=bta, in_=boxa[r0:r1])
                            nc.sync.dma_start(out=cma, in_=cmska[r0:r1])
                            nc.sync.dma_start(out=ua, in_=clsa[r0:r1])
                            nc.sync.dma_start(out=btb, in_=boxb[r0:r1])
                            nc.sync.dma_start(out=cmb_t, in_=cmskb[r0:r1])
                            nc.sync.dma_start(out=ub, in_=clsb[r0:r1])
                    if level == 0:
                        continue

                    # ---- class block: V = coo-mask AND U; G += U^T V ----
                    # AND is chunked per 32-cell block so the PE can start on
                    # block 0 while later blocks are still being masked.
                    for si, (uS, cS, dU, dC, nS, vtag) in enumerate(subs):
                        vS = tmp.tile([P, 40, nS], fp8, tag=vtag)
                        NBs = nS // 32
                        for bb in range((nS + 63) // 64):
                            c0, c1 = 64 * bb, min(64 * (bb + 1), nS)
                            cmv = cS[:, c0 // 2:c1 // 2].unsqueeze(1) \
                                .broadcast_to([P, 40, (c1 - c0) // 2])
                            V.tensor_tensor(vS[:, :, c0:c1].bitcast(u16),
                                            uS[:, :, c0:c1].bitcast(u16), cmv,
                                            op=Alu.bitwise_and)
                        if level == 6:
                            continue
                        # DoubleRow fp8: k-tile pairs are cells 16 apart (the
                        # LDW dual-fp8 ISA rule needs ktile step % 16 == 0).
                        for bb in range(NBs):
                            for j in range(16):
                                o = 32 * bb + j
                                lhs = uS[:, :, o:o + 17:16].rearrange(
                                    "p c k -> p k c", k=2)
                                rhs = vS[:, :, o:o + 17:16].rearrange(
                                    "p c k -> p k c", k=2)
                                nc.tensor.matmul(
                                    gps[0:40, 0:40], lhs, rhs,
                                    start=(it == 0 and si == 0 and bb == 0
                                           and j == 0),
                                    stop=False,
                                    perf_mode=mybir.MatmulPerfMode.DoubleRow)
                        for f in range(32 * NBs, nS):
                            nc.tensor.matmul(
                                gps[0:40, 0:40], uS[:, :, f], vS[:, :, f],
                                start=False,
                                stop=(it == T - 1 and si == 1 and f == nS - 1))
                    if level == 1 or level == 6:
                        continue

                    # ---- box block ----
                    pxy4 = bta[:, 0:4]    # cx0 cy0 cx1 cy1
                    txy4 = bta[:, 4:8]    # tx ty tx1 ty1
                    pwh4 = btb[:, 0:4]    # w0 h0 w1 h1
                    twh4 = btb[:, 4:8]    # tw th tw1 th1
                    twh01 = btb[:, 4:6]   # tw th
                    pc01 = btb[:, 8:10]   # c0 c1
                    coo = btb[:, 10]
                    tc01 = btb[:, 10:12]  # coo t9

                    scr = tmp.tile([P, 50, F], bf16, tag="scr")
                    b = it * NACC

                    twhb = twh01.rearrange("p a f -> p (a f)").unsqueeze(1) \
                        .broadcast_to([P, 2, 2 * F])
                    pwh22 = pwh4.rearrange("p (a b) f -> p a (b f)", a=2, b=2)

                    # center diffs: d00 | d11 | d10 (planes 0-5)
                    V.tensor_sub(scr[:, 0:4], pxy4, txy4)
                    V.tensor_sub(scr[:, 4:6], pxy4[:, 2:4], txy4[:, 0:2])
                    # conf diffs d49 (planes 6,7)
                    V.tensor_sub(scr[:, 6:8], pc01, tc01)

                    # av4 = |(d00,d10)|/14  (planes 8-11)
                    div = scr[:, 0:6].rearrange("p (a b) f -> p a (b f)", a=3, b=2)
                    avin = div[:, 0:3:2]
                    av4 = scr[:, 8:12].rearrange("p (a b) f -> p a (b f)", a=2, b=2)
                    A.activation(av4, avin, Act.Abs, scale=INV)

                    # s4 = wp+wt, A4 = 0.5*s4 (planes 12-15)
                    s4 = scr[:, 12:16]
                    s422 = s4.rearrange("p (a b) f -> p a (b f)", a=2, b=2)
                    V.tensor_tensor(s422, pwh22, twhb, op=Alu.add)
                    V.tensor_scalar(out=s4, in0=s4, scalar1=0.5, scalar2=None,
                                    op0=Alu.mult)
                    # oh4 = relu(min(A4-av4, wp, wt)) (planes 8-11)
                    oh4 = scr[:, 8:12]
                    oh422 = oh4.rearrange("p (a b) f -> p a (b f)", a=2, b=2)
                    V.tensor_sub(oh4, s4, oh4)
                    V.tensor_tensor(oh4, oh4, pwh4, op=Alu.min)
                    V.tensor_tensor(oh422, oh422, twhb, op=Alu.min)
                    V.tensor_scalar(out=oh4, in0=oh4, scalar1=0.0, scalar2=None,
                                    op0=Alu.max)
                    # inter (16,17), areas (18,19,20)
                    ohxy = scr[:, 8:12].rearrange("p (a b) f -> p b a f", a=2, b=2)
                    V.tensor_mul(scr[:, 16:18], ohxy[:, 0], ohxy[:, 1])
                    pwhxy = pwh4.rearrange("p (a b) f -> p b a f", a=2, b=2)
                    V.tensor_mul(scr[:, 18:20], pwhxy[:, 0], pwhxy[:, 1])
                    V.tensor_mul(scr[:, 20], btb[:, 4], btb[:, 5])
                    # union (21,22) = ap - inter + at
                    V.tensor_sub(scr[:, 21:23], scr[:, 18:20], scr[:, 16:18])
                    atb = scr[:, 20].unsqueeze(1).broadcast_to([P, 2, F])
                    V.tensor_tensor(scr[:, 21:23], scr[:, 21:23], atb, op=Alu.add)
                    # iou = inter / union: bit-magic recip + 1 Newton step
                    kb = kpl[:, :].unsqueeze(1).broadcast_to([P, 2, F])
                    r0 = scr[:, 23:25]
                    V.tensor_tensor(r0.bitcast(u16), kb,
                                    scr[:, 21:23].bitcast(u16), op=Alu.subtract)
                    tn = scr[:, 25:27]
                    V.tensor_mul(tn, scr[:, 21:23], r0)
                    V.tensor_scalar(out=tn, in0=tn, scalar1=-1.0, scalar2=2.0,
                                    op0=Alu.mult, op1=Alu.add)
                    V.tensor_mul(r0, r0, tn)
                    V.tensor_mul(scr[:, 16:18], scr[:, 16:18], r0)
                    if level == 2:
                        continue
                    # sel (27), miou (28)
                    V.tensor_tensor(scr[:, 27], scr[:, 17], scr[:, 16],
                                    op=Alu.is_gt)
                    V.tensor_tensor(scr[:, 28], scr[:, 17], scr[:, 16],
                                    op=Alu.max)
                    # masks cs0 (29), cs1 (30)
                    V.tensor_mul(scr[:, 30], coo, scr[:, 27])
                    V.tensor_sub(scr[:, 29], coo, scr[:, 30])
                    cs01 = scr[:, 29:31]
                    csdup = cs01.unsqueeze(2).broadcast_to([P, 2, 2, F])

                    # contain: (c - miou)^2 masked -> col b+1, weight 2
                    mioub = scr[:, 28].unsqueeze(1).broadcast_to([P, 2, F])
                    V.tensor_sub(scr[:, 31:33], pc01, mioub)
                    V.tensor_mul(scr[:, 33:35], cs01, scr[:, 31:33])
                    A.activation(scr[:, 31:33], scr[:, 33:35], Act.Square,
                                 scale=SQ2, accum_out=acc[:, b + 1:b + 2])
                    # not-contain: conf of non-responsible box -> col b+2, w 1
                    V.tensor_mul(scr[:, 35], scr[:, 30], btb[:, 8])
                    V.tensor_mul(scr[:, 36], scr[:, 29], btb[:, 9])
                    A.activation(scr[:, 33:35], scr[:, 35:37], Act.Square,
                                 accum_out=acc[:, b + 2:b + 3])
                    # nooobj: noo*(d4^2+d9^2) -> col b+3, weight 0.5
                    V.tensor_scalar(out=scr[:, 37], in0=coo, scalar1=-1.0,
                                    scalar2=1.0, op0=Alu.mult, op1=Alu.add)
                    noob = scr[:, 37].unsqueeze(1).broadcast_to([P, 2, F])
                    V.tensor_mul(scr[:, 6:8], noob, scr[:, 6:8])
                    A.activation(scr[:, 35:37], scr[:, 6:8], Act.Square,
                                 scale=SQH, accum_out=acc[:, b + 3:b + 4])
                    if level == 3:
                        continue

                    # loc: xy part masked (38-41)
                    dloc = scr[:, 0:4].rearrange("p (a b) f -> p a b f", a=2, b=2)
                    mdxy = scr[:, 38:42].rearrange("p (a b) f -> p a b f", a=2, b=2)
                    V.tensor_tensor(mdxy, csdup, dloc, op=Alu.mult)
                    # loc: wh part (sqrt p - sqrt t) masked (42-45)
                    A.activation(scr[:, 12:20], btb[:, 0:8], Act.Sqrt)
                    V.tensor_sub(scr[:, 12:16], scr[:, 12:16], scr[:, 16:20])
                    ds22 = scr[:, 12:16].rearrange("p (a b) f -> p a b f", a=2, b=2)
                    mds = scr[:, 42:46].rearrange("p (a b) f -> p a b f", a=2, b=2)
                    V.tensor_tensor(mds, csdup, ds22, op=Alu.mult)
                    # single masked-square accum over planes 38-45 -> col b, w 5
                    A.activation(scr[:, 12:20], scr[:, 38:46], Act.Square,
                                 scale=SQ5, accum_out=acc[:, b + 0:b + 1])

            if level >= 1 and level != 6:
                A.copy(gsb, gps)
            else:
                V.memset(gsb, 0.0)
            G.dma_start(out=out[:, :], in_=acc[:, :])
            G.dma_start(out=gout[:, :], in_=gsb)
    if fix_waits:
        _fix_multi_waits(nc)
    return nc


def _fix_multi_waits(nc):
    """Split multi-wait instructions (walrus allows one sync-wait per inst)."""
    import concourse.mybir as _mybir
    from bass_rust import SyncInfo

    blocks = [bb for fn in nc.m.functions for bb in fn.blocks]
    nseq = [0]

    def make_wait(eng, w):
        nseq[0] += 1
        ev = _mybir.InstEventSemaphore(name=f"W-split-{nseq[0]}")
        ev.engine = eng
        ev.sync_info = SyncInfo(on_wait=[w], on_update=[])
        return ev

    for bb in blocks:
        i = 0
        while i < len(bb.instructions):
            inst = bb.instructions[i]
            eng = getattr(inst, "engine", None)
            si = inst.sync_info
            if eng is None or si is None or len(si.on_wait) < 2:
                i += 1
                continue
            waits = list(si.on_wait)
            while len(waits) > 1:
                bb.instructions.insert(i, make_wait(eng, waits.pop(0)))
                i += 1
            si.on_wait[:] = waits
            i += 1


def make_in_maps(pred, target, ncores=NCORES):
    """Shard + repack host side. pred/target: [B,S,S,C] f32 np arrays."""
    bs = pred.shape[0] // ncores
    # plane index -> source (0=pred,1=target), channel
    box_src = [(0, 0), (0, 1), (0, 5), (0, 6),
               (1, 0), (1, 1), (1, 5), (1, 6),
               (0, 2), (0, 3), (0, 7), (0, 8),
               (1, 2), (1, 3), (1, 7), (1, 8),
               (0, 4), (0, 9),
               (1, 4), (1, 9)]
    in_maps = []
    for i in range(ncores):
        pf = pred[i * bs:(i + 1) * bs].reshape(-1, C)
        tf = target[i * bs:(i + 1) * bs].reshape(-1, C)
        src = (pf, tf)
        boxp = np.stack([src[s][:, c] for s, c in box_src], axis=0)  # [20, N]
        boxp = boxp.reshape(20, T * P, F).transpose(1, 0, 2).astype(BF16)
        clsp = np.concatenate([pf[:, 10:30].T, tf[:, 10:30].T], axis=0)
        clsp = clsp.reshape(40, T * P, F).transpose(1, 0, 2).astype(FP8)
        coo = (tf[:, 4] > 0)
        cm8 = np.where(coo, 0xFF, 0).astype(np.uint8)
        cm = cm8.reshape(T * P, F // 2, 2).view(np.uint16)[:, :, 0]
        in_maps.append({
            "boxa": np.ascontiguousarray(boxp[:, 0:8]),
            "boxb": np.ascontiguousarray(boxp[:, 8:20]),
            "clsa": np.ascontiguousarray(clsp[:, :, 0:192]),
            "clsb": np.ascontiguousarray(clsp[:, :, 192:]),
            "cmska": np.ascontiguousarray(cm[:, 0:96]),
            "cmskb": np.ascontiguousarray(cm[:, 96:]),
        })
    return in_maps


def combine(outs, n):
    """outs: list of per-core dicts with 'out' [P,T*NACC] and 'gout' [40,40]."""
    loc = contain = ncont = noo = cls_sum = 0.0
    for o in outs:
        a = o["out"].astype(np.float64).reshape(P, T, NACC).sum(axis=(0, 1))
        loc += a[0] / 5.0
        contain += a[1] / 2.0
        ncont += a[2]
        noo += a[3] / 0.5
        g = o["gout"].astype(np.float64)
        d = np.arange(20)
        cls_sum += (g[d, d].sum() + g[d + 20, d + 20].sum()
                    - 2.0 * g[d, d + 20].sum())
    total = (5.0 * loc + 2.0 * contain + ncont + 0.5 * noo + cls_sum) / n
    return (np.float32(total), np.float32(loc), np.float32(contain),
            np.float32(noo), np.float32(cls_sum))


_NC_CACHE = {}


def _get_nc():
    if "nc" not in _NC_CACHE:
        _NC_CACHE["nc"] = build_nc()
    return _NC_CACHE["nc"]


def run(in_maps, nc=None, **kw):
    if nc is None:
        nc = _get_nc()
    return run_bass_kernel_spmd(nc, in_maps, core_ids=list(range(len(in_maps))), **kw)


def kernel(pred, target):
    pred = np.asarray(pred, dtype=np.float32)
    target = np.asarray(target, dtype=np.float32)
    in_maps = make_in_maps(pred, target)
    res = run(in_maps)
    return combine(res.results, pred.shape[0])


if __name__ == "__main__":
    rng = np.random.default_rng(0)
    pred = rng.uniform(0.01, 1.0, (B, S, S, C)).astype(np.float32)
    target = rng.uniform(0.01, 1.0, (B, S, S, C)).astype(np.float32)
    target[..., 4] = (rng.uniform(size=(B, S, S)) < 0.1).astype(np.float32)
    print(kernel(pred, target))
